# revision 36
# baseline (speedup 1.0000x reference)
"""Trainium2 Bass kernel for LocalLuongAttention.

reference semantics (B=32, S=4096, D=1024, O=1024, STDDEV=8):
    score[b,s]  = sum_d src[b,s,d] * tgt[b,d]
    weights     = softmax(score, axis=1) * exp(-(s-pos[b])^2 / (2*8^2))
    weighted[b] = sum_s weights[b,s] * src[b,s,:]
    out         = tanh(concat([tgt, weighted], 1) @ W)        # W: [2048, 1024]

Distribution: the attention (scores/softmax/weighted sum) is data-parallel
over batch, 4 batches per core on 8 cores.  The projection is
column-parallel: each core holds only W[:, 128c:128(c+1)] (1 MB instead of
8 MB) and computes out[:, 128c:128(c+1)] for ALL 32 batches; the tiny
per-batch weighted vectors (4 KB fp32) are exchanged with one AllGather
per batch, pipelined under the next batch's streaming.  The host stitches
the column blocks and undoes the (b_local, core)-major batch ordering.

The Gaussian position decay is <= exp(-32) ~ 1.3e-14 outside +/-64 of pos,
so the weighted sum only needs a 256-row window of src (sliced on host,
kept fp32 for the window scores; the weighted sum itself runs in bf16).
The full src still streams through the chip once in bf16 for the softmax
normalizer, split across two engines to balance their roofline: s < 3072
is host-transposed to [D, S] layout and dot-multiplied on the PE
(stationary tgt column, streaming src rows, [1, 512] PSUM blocks);
s >= 3072 stays row-major and runs as accumulating STTs on the DVE with
scores in [128, 8] layout (merged into the softmax stats via gpsimd
partition reductions).
"""

import sys

for _p in ("/opt/trn_rl_repo",):
    if _p not in sys.path:
        sys.path.insert(0, _p)

from contextlib import ExitStack

import ml_dtypes
import numpy as np

import concourse.bass as bass
import concourse.tile as tile
from concourse import bacc, bass_isa, mybir
from concourse._compat import with_exitstack
from concourse.bass_utils import run_bass_kernel_spmd

B, S, D, O = 32, 4096, 1024, 1024
STDDEV = 8.0
N_CORES = 8
BPC = B // N_CORES   # batches per core
WIN = 256            # window rows kept fp32 for the window scores
HALF = 64            # guaranteed covered half-window
KC = (2 * D) // 128  # 16 contraction chunks of the projection
KD = D // 128        # 8 d-chunks of the score contraction
SP = 3072            # sequence columns scored on the PE (transposed layout)
ST = S - SP          # sequence rows scored on the DVE (row layout)
NB = SP // 512       # 512-wide PE matmul blocks per batch
NT = ST // 128       # 128-row DVE STT tiles per batch
OSH = O // N_CORES   # output columns per core

FP32 = mybir.dt.float32
BF16 = mybir.dt.bfloat16

_CACHE = {}
LAST_RESULTS = None  # BassKernelResults of the most recent run

# device batch-column order is (b_local, core)-major: column 8*b + c holds
# global batch 4*c + b
PERM = [4 * (gg % N_CORES) + gg // N_CORES for gg in range(B)]


def _install_ntff_shim():
    """Register the NTFF profile hook that this image's antenv lacks.

    Drives profiling via ctypes into libaxon_pjrt.so (same mechanism the
    full antenv.axon_hooks module uses) and stubs out the artifact upload.
    Only needed for trace=True runs.
    """
    import contextlib
    import ctypes
    import types

    if "antenv.axon_hooks" in sys.modules:
        return
    lib = ctypes.CDLL("/opt/axon/libaxon_pjrt.so")
    if not hasattr(lib, "axon_start_nrt_profile"):
        raise RuntimeError("libaxon_pjrt.so lacks profile symbols")
    lib.axon_start_nrt_profile.argtypes = [
        ctypes.POINTER(ctypes.c_int64), ctypes.c_size_t]
    lib.axon_start_nrt_profile.restype = ctypes.c_int64
    lib.axon_stop_nrt_profile.argtypes = [ctypes.c_char_p]
    lib.axon_stop_nrt_profile.restype = ctypes.c_int64

    @contextlib.contextmanager
    def _hook(output_dir, device_ids):
        import jax
        jax.devices()
        if device_ids:
            ids = (ctypes.c_int64 * len(device_ids))(*device_ids)
            rc = lib.axon_start_nrt_profile(ids, len(device_ids))
        else:
            rc = lib.axon_start_nrt_profile(None, 0)
        if rc != 0:
            raise RuntimeError(f"axon_start_nrt_profile rc={rc}")
        try:
            yield
        finally:
            n = lib.axon_stop_nrt_profile(str(output_dir).encode())
            print(f"ntff profile: {n} file(s) -> {output_dir}",
                  file=sys.stderr)

    m = types.ModuleType("antenv.axon_hooks")
    m.get_axon_ntff_profile_hook = lambda: _hook
    m.set_axon_ntff_profile_hook = lambda h: None
    sys.modules["antenv.axon_hooks"] = m
    import concourse.bass_utils as _bu
    _bu.upload_artifacts = lambda tmpdir: f"local://{tmpdir}"


@with_exitstack
def _body(ctx: ExitStack, tc: tile.TileContext, out, srcT, srcR, tgt16t,
          tgt16, tgt, tgtall, srcwin, logpw, wshard, wt_all):
    nc = tc.nc
    mult = mybir.AluOpType.mult
    maxop = mybir.AluOpType.max
    Exp = mybir.ActivationFunctionType.Exp
    Tanh = mybir.ActivationFunctionType.Tanh
    Copy = mybir.ActivationFunctionType.Copy

    consts = ctx.enter_context(tc.tile_pool(name="consts", bufs=1))
    wpool = ctx.enter_context(tc.tile_pool(name="wpool", bufs=1))
    tgtbp = ctx.enter_context(tc.tile_pool(name="tgtb", bufs=2))
    srcp = ctx.enter_context(tc.tile_pool(name="srcp", bufs=4))
    srcrp = ctx.enter_context(tc.tile_pool(name="srcrp", bufs=2))
    winp = ctx.enter_context(tc.tile_pool(name="winp", bufs=2))
    scp = ctx.enter_context(tc.tile_pool(name="scores", bufs=2))
    stats = ctx.enter_context(tc.tile_pool(name="stats", bufs=4))
    outp = ctx.enter_context(tc.tile_pool(name="outp", bufs=2))
    psc = ctx.enter_context(tc.tile_pool(name="psc", bufs=6, space="PSUM"))
    psw = ctx.enter_context(tc.tile_pool(name="psw", bufs=1, space="PSUM"))
    pso = ctx.enter_context(tc.tile_pool(name="pso", bufs=1, space="PSUM"))
    dram = ctx.enter_context(tc.tile_pool(name="dram", bufs=2, space="DRAM"))

    # Stationary tgt columns for the PE score matmuls: [128, d_chunk, batch]
    tg16 = consts.tile([128, KD, BPC], BF16)
    nc.sync.dma_start(out=tg16, in_=tgt16t.rearrange("(c p) b -> p c b",
                                                     p=128))

    # Resident projection weight shard W[:, 128c:128(c+1)]: [128, k, 128]
    wsb = wpool.tile([128, KC, OSH], FP32)

    # combined.T for ALL batches in (b_local, core)-major column order,
    # laid out [128, batch, k_chunk] so the per-batch all-gather read-back
    # is one contiguous DMA: combAt holds the tgt half (host-prearranged),
    # combAw the all-gathered weighted half.
    combAt = consts.tile([128, B, KD], FP32)
    combAw = consts.tile([128, B, KD], FP32)

    po = pso.tile([B, OSH], FP32)

    scr = consts.tile([128, D], FP32)   # discarded STT elementwise output
    zdisc = consts.tile([1, SP], FP32)  # discarded exp output
    ztdisc = consts.tile([128, NT], FP32)

    # Per-batch state carried from the stream phase to the (software-
    # pipelined) epilogue phase.
    bstate = {}

    def stream_phase(b):
        # stream DMAs go first on the sync queue so the PE never waits on
        # a batch boundary; all other bulk loads ride the scalar queue.
        srcr_ = srcT[b].rearrange("(g p) s -> p g s", p=128)
        sts = []
        for q in range(4):
            st = srcp.tile([128, KD // 4, SP], BF16, name=f"st{q}", tag="st")
            nc.sync.dma_start(out=st, in_=srcr_[:, 2 * q:2 * (q + 1), :])
            sts.append(st)
        tgtr = tgtbp.tile([1, D], FP32, tag="tgtr")
        nc.sync.dma_start(out=tgtr, in_=tgt[b:b + 1, :])
        tgtb = tgtbp.tile([128, D], FP32)
        nc.gpsimd.partition_broadcast(tgtb, tgtr)
        tgt16r = tgtbp.tile([1, D], BF16, tag="tgt16r")
        nc.sync.dma_start(out=tgt16r, in_=tgt16[b:b + 1, :])
        tgtb16 = tgtbp.tile([128, D], BF16)
        nc.gpsimd.partition_broadcast(tgtb16, tgt16r)

        # --- window path: fp32 scores on DVE, bf16 copy for the
        # weighted-sum matmuls -----------------------------------------
        winsb = winp.tile([128, 2, D], FP32)
        nc.gpsimd.dma_start(out=winsb,
                            in_=srcwin[b].rearrange("(t p) d -> p t d",
                                                    p=128))
        win16 = winp.tile([128, 2, D], BF16)
        nc.scalar.activation(win16, winsb, Copy)
        wsc = stats.tile([128, 2], FP32, tag="wsc")
        for t in range(2):
            nc.vector.scalar_tensor_tensor(
                out=scr, in0=winsb[:, t, :], scalar=0.0, in1=tgtb,
                op0=mybir.AluOpType.bypass, op1=mult,
                accum_out=wsc[:, t:t + 1])
        lpw = stats.tile([128, 2], FP32, tag="lpw")
        nc.sync.dma_start(out=lpw, in_=logpw[b])

        # --- tail rows (s >= SP) on the DVE ---------------------------
        srp = srcrp.tile([128, NT, D], BF16)
        nc.gpsimd.dma_start(out=srp,
                            in_=srcR[b].rearrange("(t p) d -> p t d", p=128))
        sct = stats.tile([128, NT], FP32, name="sct", tag="sct")
        for t in range(NT):
            nc.vector.scalar_tensor_tensor(
                out=scr, in0=srp[:, t, :], scalar=0.0, in1=tgtb16,
                op0=mybir.AluOpType.bypass, op1=mult,
                accum_out=sct[:, t:t + 1])
        mt = stats.tile([128, 1], FP32, tag="mt")
        nc.vector.tensor_reduce(mt, sct, mybir.AxisListType.X, maxop)
        nc.gpsimd.partition_all_reduce(mt, mt, 128, bass_isa.ReduceOp.max)

        # --- head columns (s < SP) on the PE --------------------------
        # scores[0, s] = sum_d srcT[d, s] * tgt[d], accumulated over the
        # 8 d-chunks into [1, 512] PSUM blocks; the scalar engine copies
        # blocks out so the DVE stays free for the STTs.
        scores = scp.tile([1, SP], FP32)
        ps = [psc.tile([1, 512], FP32, name=f"ps{j}", tag="ps")
              for j in range(NB)]
        for q in range(4):
            st = sts[q]
            for g in range(KD // 4):
                c = 2 * q + g
                for j in range(NB):
                    nc.tensor.matmul(ps[j], lhsT=tg16[:, c, b:b + 1],
                                     rhs=st[:, g, 512 * j:512 * (j + 1)],
                                     start=(c == 0), stop=(c == KD - 1),
                                     skip_group_check=True)
        for j in range(NB):
            nc.scalar.activation(scores[:, 512 * j:512 * (j + 1)], ps[j],
                                 Copy)
        ms = stats.tile([1, 1], FP32, tag="ms")
        nc.vector.tensor_reduce(ms, scores, mybir.AxisListType.X, maxop)
        bstate[b] = (tgtb, winsb, win16, wsc, lpw, sct, mt, scores, ms)

    def epilogue(b):
        (tgtb, winsb, win16, wsc, lpw, sct, mt, scores, ms) = bstate.pop(b)
        # --- softmax stats: merge PE-head and DVE-tail ----------------
        m1 = stats.tile([1, 1], FP32, tag="m1")
        nc.vector.tensor_tensor(m1, ms, mt[0:1, :], maxop)
        negm = stats.tile([1, 1], FP32, tag="negm")
        nc.vector.tensor_scalar_mul(negm, m1, -1.0)
        negmb = stats.tile([128, 1], FP32, tag="negmb")
        nc.gpsimd.partition_broadcast(negmb, negm)
        zp6 = stats.tile([1, NB], FP32, tag="zp6")
        for j in range(NB):
            nc.scalar.activation(zdisc[:, 0:512],
                                 scores[:, 512 * j:512 * (j + 1)], Exp,
                                 bias=negm, accum_out=zp6[:, j:j + 1])
        zt = stats.tile([128, 1], FP32, tag="zt")
        nc.scalar.activation(ztdisc, sct, Exp, bias=negmb, accum_out=zt)
        nc.gpsimd.partition_all_reduce(zt, zt, 128, bass_isa.ReduceOp.add)
        zp = stats.tile([1, 1], FP32, tag="zp")
        nc.vector.tensor_reduce(zp, zp6, mybir.AxisListType.X,
                                mybir.AluOpType.add)
        ztot = stats.tile([1, 1], FP32, tag="ztot")
        nc.vector.tensor_add(ztot, zp, zt[0:1, :])
        rz = stats.tile([1, 1], FP32, tag="rz")
        nc.vector.reciprocal(rz, ztot)
        rzb = stats.tile([128, 1], FP32, tag="rzb")
        nc.gpsimd.partition_broadcast(rzb, rz)

        # window weights: exp(score + logpw - m) / Z, cast to bf16
        wpre = stats.tile([128, 2], FP32, tag="wpre")
        nc.vector.tensor_add(wpre, wsc, lpw)
        wexp = stats.tile([128, 2], FP32, tag="wexp")
        nc.scalar.activation(wexp, wpre, Exp, bias=negmb)
        wfin = stats.tile([128, 2], BF16, tag="wfin")
        nc.vector.tensor_scalar_mul(wfin, wexp, rzb)

        # weighted.T chunks: contract window rows on the PE (bf16)
        combL = stats.tile([128, KD], FP32, name=f"combL{b}", tag="combL")
        for c in range(8):
            pw = psw.tile([128, 1], FP32)
            nc.tensor.matmul(pw, lhsT=win16[:, 0, 128 * c:128 * (c + 1)],
                             rhs=wfin[:, 0:1], start=True, stop=False)
            nc.tensor.matmul(pw, lhsT=win16[:, 1, 128 * c:128 * (c + 1)],
                             rhs=wfin[:, 1:2], start=False, stop=True)
            nc.vector.tensor_copy(combL[:, c:c + 1], pw)

        # all-gather this batch's weighted vector into the Shared
        # scratchpad; overlaps the next batch's streaming.  Read-backs
        # happen after the batch loop so no queue blocks on the CC.
        wt_loc = dram.tile([128, KD], FP32, name="wt_loc", tag="wt_loc")
        nc.gpsimd.dma_start(out=wt_loc, in_=combL)
        nc.gpsimd.collective_compute(
            "AllGather",
            mybir.AluOpType.bypass,
            replica_groups=[list(range(N_CORES))],
            ins=[wt_loc[:].opt()],
            outs=[wt_all[b].opt()],
        )

    for b in range(BPC):
        stream_phase(b)
        if b == 0:
            # projection constants load behind batch 0's stream traffic
            nc.scalar.dma_start(out=wsb,
                                in_=wshard.rearrange("(k p) n -> p k n",
                                                     p=128))
            nc.scalar.dma_start(out=combAt, in_=tgtall)
        if b > 0:
            epilogue(b - 1)
    # tgt half of the projection fills the PE while the last collective
    # drains; its accumulation group opens here and closes below.
    epilogue(BPC - 1)
    for k in range(KC // 2):
        nc.tensor.matmul(po, lhsT=combAt[:, :, k], rhs=wsb[:, k, :],
                         start=(k == 0), stop=False,
                         skip_group_check=True)
    for b in range(BPC):
        nc.sync.dma_start(
            out=combAw[:, N_CORES * b:N_CORES * (b + 1), :],
            in_=wt_all[b].rearrange("n p c -> p n c"))

    # weighted half of the projection closes the accumulation group
    for k in range(KC // 2, KC):
        nc.tensor.matmul(po, lhsT=combAw[:, :, k - KC // 2],
                         rhs=wsb[:, k, :],
                         start=False, stop=(k == KC - 1),
                         skip_group_check=True)
    ot = outp.tile([B, OSH], FP32)
    nc.scalar.activation(ot, po, Tanh)
    nc.sync.dma_start(out=out, in_=ot)


def build():
    if "nc" in _CACHE:
        return _CACHE["nc"]
    nc = bacc.Bacc("TRN2", target_bir_lowering=False, debug=False,
                   enable_asserts=False, num_devices=N_CORES)
    srcT = nc.dram_tensor("srcT", [BPC, D, SP], BF16,
                          kind="ExternalInput").ap()
    srcR = nc.dram_tensor("srcR", [BPC, ST, D], BF16,
                          kind="ExternalInput").ap()
    tgt16t = nc.dram_tensor("tgt16t", [D, BPC], BF16,
                            kind="ExternalInput").ap()
    tgt16 = nc.dram_tensor("tgt16", [BPC, D], BF16,
                           kind="ExternalInput").ap()
    tgt = nc.dram_tensor("tgt", [BPC, D], FP32, kind="ExternalInput").ap()
    tgtall = nc.dram_tensor("tgtall", [128, B, KD], FP32,
                            kind="ExternalInput").ap()
    srcwin = nc.dram_tensor("srcwin", [BPC, WIN, D], FP32,
                            kind="ExternalInput").ap()
    logpw = nc.dram_tensor("logpw", [BPC, 128, 2], FP32,
                           kind="ExternalInput").ap()
    wshard = nc.dram_tensor("wshard", [2 * D, OSH], FP32,
                            kind="ExternalInput").ap()
    out = nc.dram_tensor("out", [B, OSH], FP32, kind="ExternalOutput").ap()
    wt_all = nc.dram_tensor("wt_all_sh", [BPC, N_CORES, 128, KD], FP32,
                            kind="Internal", addr_space="Shared").ap()
    with tile.TileContext(nc) as tc:
        _body(tc, out, srcT, srcR, tgt16t, tgt16, tgt, tgtall, srcwin,
              logpw, wshard, wt_all)
    nc.compile()
    _CACHE["nc"] = nc
    return nc


def make_in_maps(src, tgt, pos, wmat):
    """Host-side sharding + bf16 transpose + window/log-posweight precompute."""
    src16 = src.astype(ml_dtypes.bfloat16)
    tgt16 = tgt.astype(ml_dtypes.bfloat16)
    # [128, B, KD]: tgtall[p, col, c] = tgt[PERM[col], 128c+p]
    tgtall = np.ascontiguousarray(
        tgt[PERM].reshape(B, KD, 128).transpose(2, 0, 1))
    w0 = np.clip(128 * ((pos.astype(np.int64) - HALF) // 128), 0, S - WIN)
    p_idx = np.arange(128, dtype=np.int64)[:, None]
    t_idx = np.arange(2, dtype=np.int64)[None, :]
    in_maps = []
    for c in range(N_CORES):
        bsl = slice(c * BPC, (c + 1) * BPC)
        srcwin = np.stack([
            src[c * BPC + i, w0[c * BPC + i]:w0[c * BPC + i] + WIN, :]
            for i in range(BPC)
        ])
        logpw = np.stack([
            -((w0[c * BPC + i] + t_idx * 128 + p_idx
               - pos[c * BPC + i]).astype(np.float64) ** 2)
            / (2.0 * STDDEV * STDDEV)
            for i in range(BPC)
        ]).astype(np.float32)
        in_maps.append({
            "srcT": np.ascontiguousarray(
                src16[bsl, :SP, :].transpose(0, 2, 1)),
            "srcR": np.ascontiguousarray(src16[bsl, SP:, :]),
            "tgt16t": np.ascontiguousarray(tgt16[bsl].T),
            "tgt16": np.ascontiguousarray(tgt16[bsl]),
            "tgt": np.ascontiguousarray(tgt[bsl]),
            "tgtall": tgtall,
            "srcwin": np.ascontiguousarray(srcwin),
            "logpw": logpw,
            "wshard": np.ascontiguousarray(
                wmat[:, c * OSH:(c + 1) * OSH]),
        })
    return in_maps


def kernel(source_hidden_sequence, target_hidden, positions,
           attention_weights, trace=False):
    src = np.ascontiguousarray(source_hidden_sequence, dtype=np.float32)
    tgt = np.ascontiguousarray(target_hidden, dtype=np.float32)
    pos = np.asarray(positions)
    wmat = np.ascontiguousarray(attention_weights, dtype=np.float32)
    assert src.shape == (B, S, D) and wmat.shape == (2 * D, O)

    nc = build()
    if trace:
        _install_ntff_shim()
    in_maps = make_in_maps(src, tgt, pos, wmat)
    res = run_bass_kernel_spmd(nc, in_maps, list(range(N_CORES)), trace=trace)
    global LAST_RESULTS
    LAST_RESULTS = res
    outs = np.concatenate([res.results[c]["out"] for c in range(N_CORES)],
                          axis=1)
    # undo the (b_local, core)-major device row order
    full = np.empty_like(outs)
    full[PERM] = outs
    return full.astype(np.float32)


# revision 37
# speedup vs baseline: 1.0303x; 1.0303x over previous
"""Trainium2 Bass kernel for LocalLuongAttention.

reference semantics (B=32, S=4096, D=1024, O=1024, STDDEV=8):
    score[b,s]  = sum_d src[b,s,d] * tgt[b,d]
    weights     = softmax(score, axis=1) * exp(-(s-pos[b])^2 / (2*8^2))
    weighted[b] = sum_s weights[b,s] * src[b,s,:]
    out         = tanh(concat([tgt, weighted], 1) @ W)        # W: [2048, 1024]

Distribution: the attention (scores/softmax/weighted sum) is data-parallel
over batch, 4 batches per core on 8 cores.  The projection is
column-parallel: each core holds only W[:, 128c:128(c+1)] (1 MB instead of
8 MB) and computes out[:, 128c:128(c+1)] for ALL 32 batches; the tiny
per-batch weighted vectors (4 KB fp32) are exchanged with one AllGather
per batch, pipelined under the next batch's streaming.  The host stitches
the column blocks and undoes the (b_local, core)-major batch ordering.

The Gaussian position decay is <= exp(-32) ~ 1.3e-14 outside +/-64 of pos,
so the weighted sum only needs a 256-row window of src (sliced on host,
kept fp32 for the window scores; the weighted sum itself runs in bf16).
The full src still streams through the chip once in bf16 for the softmax
normalizer, split across two engines to balance their roofline: s < 3072
is host-transposed to [D, S] layout and dot-multiplied on the PE
(stationary tgt column, streaming src rows, [1, 512] PSUM blocks);
s >= 3072 stays row-major and runs as accumulating STTs on the DVE with
scores in [128, 8] layout (merged into the softmax stats via gpsimd
partition reductions).
"""

import sys

for _p in ("/opt/trn_rl_repo",):
    if _p not in sys.path:
        sys.path.insert(0, _p)

from contextlib import ExitStack

import ml_dtypes
import numpy as np

import concourse.bass as bass
import concourse.tile as tile
from concourse import bacc, bass_isa, mybir
from concourse._compat import with_exitstack
from concourse.bass_utils import run_bass_kernel_spmd

B, S, D, O = 32, 4096, 1024, 1024
STDDEV = 8.0
N_CORES = 8
BPC = B // N_CORES   # batches per core
WIN = 256            # window rows kept fp32 for the window scores
HALF = 64            # guaranteed covered half-window
KC = (2 * D) // 128  # 16 contraction chunks of the projection
KD = D // 128        # 8 d-chunks of the score contraction
SP = 3072            # sequence columns scored on the PE (transposed layout)
ST = S - SP          # sequence rows scored on the DVE (row layout)
NB = SP // 512       # 512-wide PE matmul blocks per batch
NT = ST // 128       # 128-row DVE STT tiles per batch
OSH = O // N_CORES   # output columns per core

FP32 = mybir.dt.float32
BF16 = mybir.dt.bfloat16

_CACHE = {}
LAST_RESULTS = None  # BassKernelResults of the most recent run

# device batch-column order is (b_local, core)-major: column 8*b + c holds
# global batch 4*c + b
PERM = [4 * (gg % N_CORES) + gg // N_CORES for gg in range(B)]


def _install_ntff_shim():
    """Register the NTFF profile hook that this image's antenv lacks.

    Drives profiling via ctypes into libaxon_pjrt.so (same mechanism the
    full antenv.axon_hooks module uses) and stubs out the artifact upload.
    Only needed for trace=True runs.
    """
    import contextlib
    import ctypes
    import types

    if "antenv.axon_hooks" in sys.modules:
        return
    lib = ctypes.CDLL("/opt/axon/libaxon_pjrt.so")
    if not hasattr(lib, "axon_start_nrt_profile"):
        raise RuntimeError("libaxon_pjrt.so lacks profile symbols")
    lib.axon_start_nrt_profile.argtypes = [
        ctypes.POINTER(ctypes.c_int64), ctypes.c_size_t]
    lib.axon_start_nrt_profile.restype = ctypes.c_int64
    lib.axon_stop_nrt_profile.argtypes = [ctypes.c_char_p]
    lib.axon_stop_nrt_profile.restype = ctypes.c_int64

    @contextlib.contextmanager
    def _hook(output_dir, device_ids):
        import jax
        jax.devices()
        if device_ids:
            ids = (ctypes.c_int64 * len(device_ids))(*device_ids)
            rc = lib.axon_start_nrt_profile(ids, len(device_ids))
        else:
            rc = lib.axon_start_nrt_profile(None, 0)
        if rc != 0:
            raise RuntimeError(f"axon_start_nrt_profile rc={rc}")
        try:
            yield
        finally:
            n = lib.axon_stop_nrt_profile(str(output_dir).encode())
            print(f"ntff profile: {n} file(s) -> {output_dir}",
                  file=sys.stderr)

    m = types.ModuleType("antenv.axon_hooks")
    m.get_axon_ntff_profile_hook = lambda: _hook
    m.set_axon_ntff_profile_hook = lambda h: None
    sys.modules["antenv.axon_hooks"] = m
    import concourse.bass_utils as _bu
    _bu.upload_artifacts = lambda tmpdir: f"local://{tmpdir}"


@with_exitstack
def _body(ctx: ExitStack, tc: tile.TileContext, out, srcT, srcR, tgt16t,
          tgt16, tgt, tgtall, srcwin, logpw, wshard, wt_all):
    nc = tc.nc
    mult = mybir.AluOpType.mult
    maxop = mybir.AluOpType.max
    Exp = mybir.ActivationFunctionType.Exp
    Tanh = mybir.ActivationFunctionType.Tanh
    Copy = mybir.ActivationFunctionType.Copy

    consts = ctx.enter_context(tc.tile_pool(name="consts", bufs=1))
    wpool = ctx.enter_context(tc.tile_pool(name="wpool", bufs=1))
    tgtbp = ctx.enter_context(tc.tile_pool(name="tgtb", bufs=2))
    srcp = ctx.enter_context(tc.tile_pool(name="srcp", bufs=4))
    srcrp = ctx.enter_context(tc.tile_pool(name="srcrp", bufs=2))
    winp = ctx.enter_context(tc.tile_pool(name="winp", bufs=2))
    scp = ctx.enter_context(tc.tile_pool(name="scores", bufs=2))
    stats = ctx.enter_context(tc.tile_pool(name="stats", bufs=4))
    outp = ctx.enter_context(tc.tile_pool(name="outp", bufs=2))
    psc = ctx.enter_context(tc.tile_pool(name="psc", bufs=6, space="PSUM"))
    psw = ctx.enter_context(tc.tile_pool(name="psw", bufs=1, space="PSUM"))
    pso = ctx.enter_context(tc.tile_pool(name="pso", bufs=1, space="PSUM"))
    dram = ctx.enter_context(tc.tile_pool(name="dram", bufs=2, space="DRAM"))

    # Stationary tgt columns for the PE score matmuls: [128, d_chunk, batch]
    tg16 = consts.tile([128, KD, BPC], BF16)
    nc.sync.dma_start(out=tg16, in_=tgt16t.rearrange("(c p) b -> p c b",
                                                     p=128))

    # Resident projection weight shard W[:, 128c:128(c+1)]: [128, k, 128]
    wsb = wpool.tile([128, KC, OSH], FP32)
    nc.sync.dma_start(out=wsb, in_=wshard.rearrange("(k p) n -> p k n",
                                                    p=128))

    # combined.T for ALL batches in (b_local, core)-major column order,
    # laid out [128, batch, k_chunk] so the per-batch all-gather read-back
    # is one contiguous DMA: combAt holds the tgt half (host-prearranged),
    # combAw the all-gathered weighted half.
    combAt = consts.tile([128, B, KD], FP32)
    nc.sync.dma_start(out=combAt, in_=tgtall)
    combAw = consts.tile([128, B, KD], FP32)

    po = pso.tile([B, OSH], FP32)

    scr = consts.tile([128, D], FP32)   # discarded STT elementwise output
    zdisc = consts.tile([1, SP], FP32)  # discarded exp output
    ztdisc = consts.tile([128, NT], FP32)

    # Per-batch state carried from the stream phase to the (software-
    # pipelined) epilogue phase.
    bstate = {}

    def stream_phase(b):
        tgtr = tgtbp.tile([1, D], FP32, tag="tgtr")
        nc.sync.dma_start(out=tgtr, in_=tgt[b:b + 1, :])
        tgtb = tgtbp.tile([128, D], FP32)
        nc.gpsimd.partition_broadcast(tgtb, tgtr)
        tgt16r = tgtbp.tile([1, D], BF16, tag="tgt16r")
        nc.sync.dma_start(out=tgt16r, in_=tgt16[b:b + 1, :])
        tgtb16 = tgtbp.tile([128, D], BF16)
        nc.gpsimd.partition_broadcast(tgtb16, tgt16r)

        # --- window path: fp32 scores on DVE, bf16 copy for the
        # weighted-sum matmuls -----------------------------------------
        winsb = winp.tile([128, 2, D], FP32)
        nc.sync.dma_start(out=winsb,
                          in_=srcwin[b].rearrange("(t p) d -> p t d", p=128))
        win16 = winp.tile([128, 2, D], BF16)
        nc.scalar.activation(win16, winsb, Copy)
        wsc = stats.tile([128, 2], FP32, tag="wsc")
        for t in range(2):
            nc.vector.scalar_tensor_tensor(
                out=scr, in0=winsb[:, t, :], scalar=0.0, in1=tgtb,
                op0=mybir.AluOpType.bypass, op1=mult,
                accum_out=wsc[:, t:t + 1])
        lpw = stats.tile([128, 2], FP32, tag="lpw")
        nc.sync.dma_start(out=lpw, in_=logpw[b])

        # --- tail rows (s >= SP) on the DVE ---------------------------
        srp = srcrp.tile([128, NT, D], BF16)
        nc.sync.dma_start(out=srp,
                          in_=srcR[b].rearrange("(t p) d -> p t d", p=128))
        sct = stats.tile([128, NT], FP32, name="sct", tag="sct")
        for t in range(NT):
            nc.vector.scalar_tensor_tensor(
                out=scr, in0=srp[:, t, :], scalar=0.0, in1=tgtb16,
                op0=mybir.AluOpType.bypass, op1=mult,
                accum_out=sct[:, t:t + 1])
        mt = stats.tile([128, 1], FP32, tag="mt")
        nc.vector.tensor_reduce(mt, sct, mybir.AxisListType.X, maxop)
        nc.gpsimd.partition_all_reduce(mt, mt, 128, bass_isa.ReduceOp.max)

        # --- head columns (s < SP) on the PE --------------------------
        # scores[0, s] = sum_d srcT[d, s] * tgt[d], accumulated over the
        # 8 d-chunks into [1, 512] PSUM blocks; the scalar engine copies
        # blocks out so the DVE stays free for the STTs.
        scores = scp.tile([1, SP], FP32)
        srcr_ = srcT[b].rearrange("(g p) s -> p g s", p=128)
        ps = [psc.tile([1, 512], FP32, name=f"ps{j}", tag="ps")
              for j in range(NB)]
        for q in range(4):
            st = srcp.tile([128, KD // 4, SP], BF16, name=f"st{q}", tag="st")
            nc.sync.dma_start(out=st, in_=srcr_[:, 2 * q:2 * (q + 1), :])
            for g in range(KD // 4):
                c = 2 * q + g
                for j in range(NB):
                    nc.tensor.matmul(ps[j], lhsT=tg16[:, c, b:b + 1],
                                     rhs=st[:, g, 512 * j:512 * (j + 1)],
                                     start=(c == 0), stop=(c == KD - 1),
                                     skip_group_check=True)
        for j in range(NB):
            nc.scalar.activation(scores[:, 512 * j:512 * (j + 1)], ps[j],
                                 Copy)
        ms = stats.tile([1, 1], FP32, tag="ms")
        nc.vector.tensor_reduce(ms, scores, mybir.AxisListType.X, maxop)
        bstate[b] = (tgtb, winsb, win16, wsc, lpw, sct, mt, scores, ms)

    def epilogue(b):
        (tgtb, winsb, win16, wsc, lpw, sct, mt, scores, ms) = bstate.pop(b)
        # --- softmax stats: merge PE-head and DVE-tail ----------------
        m1 = stats.tile([1, 1], FP32, tag="m1")
        nc.vector.tensor_tensor(m1, ms, mt[0:1, :], maxop)
        negm = stats.tile([1, 1], FP32, tag="negm")
        nc.vector.tensor_scalar_mul(negm, m1, -1.0)
        negmb = stats.tile([128, 1], FP32, tag="negmb")
        nc.gpsimd.partition_broadcast(negmb, negm)
        zp6 = stats.tile([1, NB], FP32, tag="zp6")
        for j in range(NB):
            nc.scalar.activation(zdisc[:, 0:512],
                                 scores[:, 512 * j:512 * (j + 1)], Exp,
                                 bias=negm, accum_out=zp6[:, j:j + 1])
        zt = stats.tile([128, 1], FP32, tag="zt")
        nc.scalar.activation(ztdisc, sct, Exp, bias=negmb, accum_out=zt)
        nc.gpsimd.partition_all_reduce(zt, zt, 128, bass_isa.ReduceOp.add)
        zp = stats.tile([1, 1], FP32, tag="zp")
        nc.vector.tensor_reduce(zp, zp6, mybir.AxisListType.X,
                                mybir.AluOpType.add)
        ztot = stats.tile([1, 1], FP32, tag="ztot")
        nc.vector.tensor_add(ztot, zp, zt[0:1, :])
        rz = stats.tile([1, 1], FP32, tag="rz")
        nc.vector.reciprocal(rz, ztot)
        rzb = stats.tile([128, 1], FP32, tag="rzb")
        nc.gpsimd.partition_broadcast(rzb, rz)

        # window weights: exp(score + logpw - m) / Z, cast to bf16
        wpre = stats.tile([128, 2], FP32, tag="wpre")
        nc.vector.tensor_add(wpre, wsc, lpw)
        wexp = stats.tile([128, 2], FP32, tag="wexp")
        nc.scalar.activation(wexp, wpre, Exp, bias=negmb)
        wfin = stats.tile([128, 2], BF16, tag="wfin")
        nc.vector.tensor_scalar_mul(wfin, wexp, rzb)

        # weighted.T chunks: contract window rows on the PE (bf16)
        combL = stats.tile([128, KD], FP32, name=f"combL{b}", tag="combL")
        for c in range(8):
            pw = psw.tile([128, 1], FP32)
            nc.tensor.matmul(pw, lhsT=win16[:, 0, 128 * c:128 * (c + 1)],
                             rhs=wfin[:, 0:1], start=True, stop=False)
            nc.tensor.matmul(pw, lhsT=win16[:, 1, 128 * c:128 * (c + 1)],
                             rhs=wfin[:, 1:2], start=False, stop=True)
            nc.vector.tensor_copy(combL[:, c:c + 1], pw)

        # all-gather this batch's weighted vector into the Shared
        # scratchpad; overlaps the next batch's streaming.  Read-backs
        # happen after the batch loop so no queue blocks on the CC.
        wt_loc = dram.tile([128, KD], FP32, name="wt_loc", tag="wt_loc")
        nc.gpsimd.dma_start(out=wt_loc, in_=combL)
        nc.gpsimd.collective_compute(
            "AllGather",
            mybir.AluOpType.bypass,
            replica_groups=[list(range(N_CORES))],
            ins=[wt_loc[:].opt()],
            outs=[wt_all[b].opt()],
        )

    for b in range(BPC):
        stream_phase(b)
        if b > 0:
            epilogue(b - 1)
    # tgt half of the projection fills the PE while the last collective
    # drains; its accumulation group opens here and closes below.
    epilogue(BPC - 1)
    for k in range(KC // 2):
        nc.tensor.matmul(po, lhsT=combAt[:, :, k], rhs=wsb[:, k, :],
                         start=(k == 0), stop=False,
                         skip_group_check=True)
    for b in range(BPC):
        nc.sync.dma_start(
            out=combAw[:, N_CORES * b:N_CORES * (b + 1), :],
            in_=wt_all[b].rearrange("n p c -> p n c"))

    # weighted half of the projection closes the accumulation group
    for k in range(KC // 2, KC):
        nc.tensor.matmul(po, lhsT=combAw[:, :, k - KC // 2],
                         rhs=wsb[:, k, :],
                         start=False, stop=(k == KC - 1),
                         skip_group_check=True)
    ot = outp.tile([B, OSH], FP32)
    nc.scalar.activation(ot, po, Tanh)
    nc.sync.dma_start(out=out, in_=ot)


def build():
    if "nc" in _CACHE:
        return _CACHE["nc"]
    nc = bacc.Bacc("TRN2", target_bir_lowering=False, debug=False,
                   enable_asserts=False, num_devices=N_CORES)
    srcT = nc.dram_tensor("srcT", [BPC, D, SP], BF16,
                          kind="ExternalInput").ap()
    srcR = nc.dram_tensor("srcR", [BPC, ST, D], BF16,
                          kind="ExternalInput").ap()
    tgt16t = nc.dram_tensor("tgt16t", [D, BPC], BF16,
                            kind="ExternalInput").ap()
    tgt16 = nc.dram_tensor("tgt16", [BPC, D], BF16,
                           kind="ExternalInput").ap()
    tgt = nc.dram_tensor("tgt", [BPC, D], FP32, kind="ExternalInput").ap()
    tgtall = nc.dram_tensor("tgtall", [128, B, KD], FP32,
                            kind="ExternalInput").ap()
    srcwin = nc.dram_tensor("srcwin", [BPC, WIN, D], FP32,
                            kind="ExternalInput").ap()
    logpw = nc.dram_tensor("logpw", [BPC, 128, 2], FP32,
                           kind="ExternalInput").ap()
    wshard = nc.dram_tensor("wshard", [2 * D, OSH], FP32,
                            kind="ExternalInput").ap()
    out = nc.dram_tensor("out", [B, OSH], FP32, kind="ExternalOutput").ap()
    wt_all = nc.dram_tensor("wt_all_sh", [BPC, N_CORES, 128, KD], FP32,
                            kind="Internal", addr_space="Shared").ap()
    with tile.TileContext(nc) as tc:
        _body(tc, out, srcT, srcR, tgt16t, tgt16, tgt, tgtall, srcwin,
              logpw, wshard, wt_all)
    nc.compile()
    _CACHE["nc"] = nc
    return nc


def make_in_maps(src, tgt, pos, wmat):
    """Host-side sharding + bf16 transpose + window/log-posweight precompute."""
    src16 = src.astype(ml_dtypes.bfloat16)
    tgt16 = tgt.astype(ml_dtypes.bfloat16)
    # [128, B, KD]: tgtall[p, col, c] = tgt[PERM[col], 128c+p]
    tgtall = np.ascontiguousarray(
        tgt[PERM].reshape(B, KD, 128).transpose(2, 0, 1))
    w0 = np.clip(128 * ((pos.astype(np.int64) - HALF) // 128), 0, S - WIN)
    p_idx = np.arange(128, dtype=np.int64)[:, None]
    t_idx = np.arange(2, dtype=np.int64)[None, :]
    in_maps = []
    for c in range(N_CORES):
        bsl = slice(c * BPC, (c + 1) * BPC)
        srcwin = np.stack([
            src[c * BPC + i, w0[c * BPC + i]:w0[c * BPC + i] + WIN, :]
            for i in range(BPC)
        ])
        logpw = np.stack([
            -((w0[c * BPC + i] + t_idx * 128 + p_idx
               - pos[c * BPC + i]).astype(np.float64) ** 2)
            / (2.0 * STDDEV * STDDEV)
            for i in range(BPC)
        ]).astype(np.float32)
        in_maps.append({
            "srcT": np.ascontiguousarray(
                src16[bsl, :SP, :].transpose(0, 2, 1)),
            "srcR": np.ascontiguousarray(src16[bsl, SP:, :]),
            "tgt16t": np.ascontiguousarray(tgt16[bsl].T),
            "tgt16": np.ascontiguousarray(tgt16[bsl]),
            "tgt": np.ascontiguousarray(tgt[bsl]),
            "tgtall": tgtall,
            "srcwin": np.ascontiguousarray(srcwin),
            "logpw": logpw,
            "wshard": np.ascontiguousarray(
                wmat[:, c * OSH:(c + 1) * OSH]),
        })
    return in_maps


def kernel(source_hidden_sequence, target_hidden, positions,
           attention_weights, trace=False):
    src = np.ascontiguousarray(source_hidden_sequence, dtype=np.float32)
    tgt = np.ascontiguousarray(target_hidden, dtype=np.float32)
    pos = np.asarray(positions)
    wmat = np.ascontiguousarray(attention_weights, dtype=np.float32)
    assert src.shape == (B, S, D) and wmat.shape == (2 * D, O)

    nc = build()
    if trace:
        _install_ntff_shim()
    in_maps = make_in_maps(src, tgt, pos, wmat)
    res = run_bass_kernel_spmd(nc, in_maps, list(range(N_CORES)), trace=trace)
    global LAST_RESULTS
    LAST_RESULTS = res
    outs = np.concatenate([res.results[c]["out"] for c in range(N_CORES)],
                          axis=1)
    # undo the (b_local, core)-major device row order
    full = np.empty_like(outs)
    full[PERM] = outs
    return full.astype(np.float32)
